# revision 11
# baseline (speedup 1.0000x reference)
"""Trainium2 Bass kernel for nn_CenterRegressor (4-layer GraphSAGE, mean-agg).

Self-contained: takes FULL inputs, shards across 8 NeuronCores internally,
returns the FULL [50000, 3] float32 output.

Design (per core, nodes sharded 8 ways, N padded 50000->50176):
  - per layer: AllGather h slices -> per-pair HBM replica h_rep (bf16)
  - edge-gather: dma_gather of 512B bf16 rows (4 SWDGE queues), edges sorted
    by (dst-tile, src-half), chunk counts padded to the max across cores so
    all 8 cores run one SPMD program
  - segment-mean: fp8 one-hot [128 slots x 128 dst] matmuls accumulate in
    PSUM; inv-degree folded into the PSUM->SBUF copy (ScalarE, per-partition
    scale); pad slots have all-zero one-hot columns
  - dense: out = agg@Wl + bl + h@Wr via DMA-transposed activations as
    stationary operands; biases as K=1 matmuls
  - epilogue: bn_stats/bn_aggr per tile + batched per-layer scalar math
    (L2-normalize + LayerNorm fold into one row-affine), SiLU on ScalarE,
    residual on VectorE
"""
import os
import sys
import types
import contextlib

import numpy as np

sys.path.insert(0, "/opt/trn_rl_repo")

import ml_dtypes  # noqa: E402
import concourse.bacc as bacc  # noqa: E402
import concourse.bass as bass  # noqa: E402
import concourse.mybir as mybir  # noqa: E402
import concourse.tile as tile  # noqa: E402
from concourse.bass_utils import run_bass_kernel_spmd  # noqa: E402
from concourse.library_config import mlp  # noqa: E402

BF16 = ml_dtypes.bfloat16
FP8 = ml_dtypes.float8_e4m3fn
AF = mybir.ActivationFunctionType

LN_EPS = 1e-5
L2_EPS = 1e-12
MAX_CHUNKS_PER_CALL = 8  # <=1024 idxs per dma_gather call


def _enable_axon_profile():
    if "antenv.axon_hooks" not in sys.modules:
        mod = types.ModuleType("antenv.axon_hooks")
        holder = [None]
        mod.set_axon_ntff_profile_hook = lambda h: holder.__setitem__(0, h)
        mod.get_axon_ntff_profile_hook = lambda: holder[0]
        sys.modules["antenv.axon_hooks"] = mod
        import antenv
        antenv.axon_hooks = mod
        try:
            from trn_agent_boot.trn_boot import _ntff_profile_via_ctypes
            mod.set_axon_ntff_profile_hook(
                _ntff_profile_via_ctypes("/opt/axon/libaxon_pjrt.so"))
        except Exception:
            pass
    import concourse.bass_utils as bu
    bu.upload_artifacts = lambda tmpdir: f"file://{tmpdir}"


def _idx_layout(flat):
    """int16 [S] -> [128, S/16] wrapped in 16 partitions, replicated x8."""
    s = flat.shape[0]
    assert s % 16 == 0
    return np.tile(flat.reshape(s // 16, 16).T, (8, 1)).astype(np.int16)


def preprocess(edge_index, n_nodes, W, npc):
    """Build the uniform SPMD schedule + per-core gather/one-hot data.

    Returns (sched, percore) where sched is compile-time (identical for all
    cores) and percore is a list of dicts of numpy arrays.
    """
    npad = W * npc
    halfr = npad // 2
    T = npc // 128
    src = np.asarray(edge_index[0], dtype=np.int64)
    dst = np.asarray(edge_index[1], dtype=np.int64)

    deg = np.bincount(dst, minlength=npad).astype(np.float64)
    inv_deg = (1.0 / np.maximum(deg, 1.0)).astype(np.float32)

    core_of = dst // npc
    # per (core, tile, half) edge lists
    counts = np.zeros((W, T, 2), dtype=np.int64)
    per_cth = {}
    for c in range(W):
        m = core_of == c
        s_c = src[m]
        dl = dst[m] - c * npc
        t_c = dl // 128
        p_c = dl % 128
        hf_c = (s_c >= halfr).astype(np.int64)
        key = t_c * 2 + hf_c
        order = np.argsort(key, kind="stable")
        s_c, p_c, t_c, hf_c, key = (a[order] for a in (s_c, p_c, t_c, hf_c, key))
        bounds = np.searchsorted(key, np.arange(2 * T + 1))
        for t in range(T):
            for hf in range(2):
                lo, hi = bounds[t * 2 + hf], bounds[t * 2 + hf + 1]
                per_cth[(c, t, hf)] = (s_c[lo:hi] - hf * halfr, p_c[lo:hi])
                counts[c, t, hf] = hi - lo

    # uniform chunk counts across cores
    K = np.maximum(np.ceil(counts / 128.0).astype(np.int64).max(axis=0), 0)
    K[:, 0] = np.maximum(K[:, 0], 1)  # >=1 chunk so PSUM gets a start matmul

    # global chunk order: all lo chunks (by tile), then all hi chunks (by tile)
    # cth_off[hf][t] = first global chunk index of (t, hf)
    nlo = int(K[:, 0].sum())
    nhi = int(K[:, 1].sum())
    total_chunks = nlo + nhi
    total_slots = total_chunks * 128
    cth_off = np.zeros((2, T + 1), dtype=np.int64)
    cth_off[0, 1:] = np.cumsum(K[:, 0])
    cth_off[1, 1:] = nlo + np.cumsum(K[:, 1])
    cth_off[1, 0] = nlo

    # calls: runs of <=8 chunks within each half, spanning tiles
    calls = []  # (chunk0, n_chunks, half)
    for hf in range(2):
        lo, hi = (0, nlo) if hf == 0 else (nlo, nlo + nhi)
        c = lo
        while c < hi:
            n = min(MAX_CHUNKS_PER_CALL, hi - c)
            calls.append((c, n, hf))
            c += n

    sched = dict(W=W, npc=npc, npad=npad, halfr=halfr, T=T,
                 K=K, cth_off=cth_off, total_chunks=total_chunks,
                 total_slots=total_slots, calls=calls)

    percore = []
    for c in range(W):
        idx_flat = np.zeros(total_slots, dtype=np.int16)
        oh = np.zeros((128, total_chunks * 128), dtype=FP8)
        for t in range(T):
            for hf in range(2):
                srel, p = per_cth[(c, t, hf)]
                off = int(cth_off[hf, t]) * 128
                n = srel.shape[0]
                idx_flat[off:off + n] = srel.astype(np.int16)
                sl = np.arange(n) + off
                # slot s lives at partition s%128, chunk s//128
                oh[sl % 128, (sl // 128) * 128 + p] = 1.0
        inv_sb = inv_deg[c * npc:(c + 1) * npc].reshape(T, 128).T.copy()  # [128,T]
        percore.append(dict(idx=_idx_layout(idx_flat), oh=oh, inv=inv_sb))
    return sched, percore


def build_program(sched, L, in_dim, H):
    W, npc, T = sched["W"], sched["npc"], sched["T"]
    npad, halfr = sched["npad"], sched["halfr"]
    K, cth_off, calls = sched["K"], sched["cth_off"], sched["calls"]
    total_chunks, total_slots = sched["total_chunks"], sched["total_slots"]
    KC = H // 128  # feature chunks (2)
    ohmax = int(max((sched["K"][t, 0] + sched["K"][t, 1]) for t in range(sched["T"])))

    nc = bacc.Bacc("TRN2", debug=True, num_swdge_queues=4)
    f32, bf16, fp8, i16 = (mybir.dt.float32, mybir.dt.bfloat16,
                           mybir.dt.float8e4, mybir.dt.int16)

    # ---- external IO ----
    xT_in = nc.dram_tensor("xT", [in_dim, npc], bf16, kind="ExternalInput")
    idx_in = nc.dram_tensor("idx", [128, total_slots // 16], i16, kind="ExternalInput")
    oh_in = nc.dram_tensor("oh", [128, total_chunks * 128], fp8, kind="ExternalInput")
    inv_in = nc.dram_tensor("inv", [128, T], f32, kind="ExternalInput")
    wp_in = nc.dram_tensor("wp", [in_dim, H], bf16, kind="ExternalInput")
    wl_in = nc.dram_tensor("wl", [128, L * KC, H], bf16, kind="ExternalInput")
    wr_in = nc.dram_tensor("wr", [128, L * KC, H], bf16, kind="ExternalInput")
    w1_in = nc.dram_tensor("w1", [128, KC, H], bf16, kind="ExternalInput")
    w2_in = nc.dram_tensor("w2", [128, KC, 3], bf16, kind="ExternalInput")
    bias_in = nc.dram_tensor("biases", [1, (L + 2) * H + 3], bf16, kind="ExternalInput")
    gb_in = nc.dram_tensor("gb", [128, 2 * L * H], bf16, kind="ExternalInput")
    ones_in = nc.dram_tensor("ones", [1, 128], bf16, kind="ExternalInput")
    ident_in = nc.dram_tensor("ident", [128, 128], bf16, kind="ExternalInput")
    out_ext = nc.dram_tensor("out", [npc, 3], f32, kind="ExternalOutput")

    # ---- internal DRAM ----
    bounce = nc.dram_tensor("bounce", [npc, H], bf16)
    h_rep = nc.dram_tensor("h_rep", [npad, H], bf16,
                           addr_space="Shared" if W > 1 else "Local")

    with contextlib.ExitStack() as ctx:
        tc = ctx.enter_context(tile.TileContext(nc))
        const = ctx.enter_context(tc.tile_pool(name="const", bufs=1))
        gpool = ctx.enter_context(tc.tile_pool(name="gath", bufs=16))
        ohpool = ctx.enter_context(tc.tile_pool(name="ohp", bufs=4))
        work = ctx.enter_context(tc.tile_pool(name="work", bufs=4))
        pa = ctx.enter_context(tc.tile_pool(name="pa", bufs=2, space="PSUM"))
        pt = ctx.enter_context(tc.tile_pool(name="pt", bufs=3, space="PSUM"))
        po = ctx.enter_context(tc.tile_pool(name="po", bufs=3, space="PSUM"))

        nc.gpsimd.load_library(mlp)

        def load_const(name, dram, shape, dt):
            t = const.tile(shape, dt, tag=name)
            nc.sync.dma_start(t[:], dram[:])
            return t

        idx_sb = load_const("idx", idx_in, [128, total_slots // 16], i16)
        inv_sb = load_const("inv", inv_in, [128, T], f32)
        xT_sb = load_const("xT", xT_in, [in_dim, npc], bf16)
        wp_sb = load_const("wp", wp_in, [in_dim, H], bf16)
        wl_sb = load_const("wl", wl_in, [128, L * KC, H], bf16)
        wr_sb = load_const("wr", wr_in, [128, L * KC, H], bf16)
        w1_sb = load_const("w1", w1_in, [128, KC, H], bf16)
        w2_sb = load_const("w2", w2_in, [128, KC, 3], bf16)
        bias_sb = load_const("biases", bias_in, [1, (L + 2) * H + 3], bf16)
        gb_sb = load_const("gb", gb_in, [128, 2 * L * H], bf16)
        ones_sb = load_const("ones", ones_in, [1, 128], bf16)
        ident_sb = load_const("ident", ident_in, [128, 128], bf16)

        zero1 = const.tile([128, 1], f32, tag="zero1")
        nc.vector.memset(zero1[:], 0.0)
        eps1 = const.tile([128, 1], f32, tag="eps1")
        nc.vector.memset(eps1[:], LN_EPS)

        h_c = const.tile([128, T, H], bf16, tag="h_c")
        out_buf = const.tile([128, T, H], bf16, tag="out_buf")
        stats6 = const.tile([128, T, 6], f32, tag="stats6")
        stats2 = const.tile([128, T, 2], f32, tag="stats2")
        smA = const.tile([128, T], f32, tag="smA")
        smB = const.tile([128, T], f32, tag="smB")
        Tt = const.tile([128, T], f32, tag="Tt")
        Bb = const.tile([128, T], f32, tag="Bb")

        def bias_ap(which):
            # biases layout: [bp(H) | bl0..bl3 (L*H) | b1(H) | b2(3)]
            if which == "bp":
                return bias_sb[:, 0:H]
            if which.startswith("bl"):
                l = int(which[2:])
                return bias_sb[:, H + l * H: H + (l + 1) * H]
            if which == "b1":
                return bias_sb[:, (L + 1) * H:(L + 2) * H]
            return bias_sb[:, (L + 2) * H:(L + 2) * H + 3]

        # issue gather calls lazily; chunk c served by call c//8 slot c%8
        gt_bufs = {}

        def ensure_call(ci):
            if ci in gt_bufs:
                return gt_bufs[ci]
            (c0, nch, hf) = calls[ci]
            gt = gpool.tile([128, MAX_CHUNKS_PER_CALL, H], bf16, tag="gt")
            nidx = nch * 128
            slot_off = c0 * 128
            src_ap = h_rep[0:halfr, :] if hf == 0 else h_rep[halfr:npad, :]
            nc.gpsimd.dma_gather(
                gt[:, 0:nch, :], src_ap,
                idx_sb[:, slot_off // 16:(slot_off + nidx) // 16],
                nidx, nidx, H,
                single_packet=False, queue_num=ci % 4)
            gt_bufs[ci] = gt
            return gt

        call_of_chunk = {}
        for ci, (c0, nch, hf) in enumerate(calls):
            for j in range(nch):
                call_of_chunk[c0 + j] = (ci, j)

        def agg_tile(t):
            """Gather + one-hot matmul + inv-deg scale -> agg [128,H] bf16."""
            chunks = (list(range(int(cth_off[0, t]), int(cth_off[0, t + 1]))) +
                      list(range(int(cth_off[1, t]), int(cth_off[1, t + 1]))))
            ktot = len(chunks)
            oh_t = ohpool.tile([128, ohmax * 128], fp8, tag="oh")
            lo0, lo1 = int(cth_off[0, t]), int(cth_off[0, t + 1])
            hi0, hi1 = int(cth_off[1, t]), int(cth_off[1, t + 1])
            nlo_t = lo1 - lo0
            nc.sync.dma_start(oh_t[:, 0:nlo_t * 128], oh_in[:, lo0 * 128:lo1 * 128])
            nc.sync.dma_start(oh_t[:, nlo_t * 128:ktot * 128],
                              oh_in[:, hi0 * 128:hi1 * 128])
            psum_a = pa.tile([128, H], f32, tag="psa")
            for i, ch in enumerate(chunks):
                ci, j = call_of_chunk[ch]
                gt = ensure_call(ci)
                nc.tensor.matmul(
                    psum_a[:], oh_t[:, i * 128:(i + 1) * 128], gt[:, j, :],
                    start=(i == 0), stop=(i == ktot - 1))
            agg = work.tile([128, H], bf16, tag="agg")
            nc.scalar.activation(agg[:], psum_a[:], AF.Copy, scale=inv_sb[:, t:t + 1])
            return agg

        def transpose2(src_ap, tag):
            tt = work.tile([128, KC, 128], bf16, tag=tag)
            for k in range(KC):
                pst = pt.tile([128, 128], bf16, tag="pst")
                nc.tensor.transpose(pst[:], src_ap[:, k * 128:(k + 1) * 128],
                                    ident_sb[:])
                if k % 2 == 0:
                    nc.scalar.activation(tt[:, k, :], pst[:], AF.Copy)
                else:
                    nc.vector.tensor_copy(tt[:, k, :], pst[:])
            return tt

        AGQ = 4 if T % 4 == 0 else 1
        h_rep_v = h_rep[:].rearrange("(w n) h -> w n h", w=W)

        def allgather():
            if W == 1:
                nc.sync.dma_start(h_rep[:], bounce[:])
                return
            q = npc // AGQ
            for i in range(AGQ):
                nc.gpsimd.collective_compute(
                    "AllGather", mybir.AluOpType.bypass,
                    replica_groups=[list(range(W))],
                    ins=[bounce[i * q:(i + 1) * q, :].opt()],
                    outs=[h_rep_v[:, i * q:(i + 1) * q, :].opt()])

        # ---- input projection ----
        for t in range(T):
            psum_o = po.tile([128, H], f32, tag="pso")
            nc.tensor.matmul(psum_o[:], ones_sb[:], bias_ap("bp"),
                             start=True, stop=False)
            nc.tensor.matmul(psum_o[:], xT_sb[:, t * 128:(t + 1) * 128], wp_sb[:],
                             start=False, stop=True)
            nc.scalar.activation(h_c[:, t, :], psum_o[:], AF.Copy)
            nc.sync.dma_start(bounce[t * 128:(t + 1) * 128, :], h_c[:, t, :])

        # ---- layers ----
        for l in range(L):
            gt_bufs.clear()  # per-layer gather-call memo
            with nc.named_scope(f"ag{l}"):
                allgather()
            # phase H: AllGather-independent dense part (bias + h@Wr)
            for t in range(T):
                hT = transpose2(h_c[:, t, :], "hT")
                psum_h = po.tile([128, H], f32, tag="pso")
                nc.tensor.matmul(psum_h[:], ones_sb[:], bias_ap(f"bl{l}"),
                                 start=True, stop=False)
                for k in range(KC):
                    nc.tensor.matmul(psum_h[:], hT[:, k, :], wr_sb[:, l * KC + k, :],
                                     start=False, stop=(k == KC - 1))
                nc.vector.tensor_copy(out_buf[:, t, :], psum_h[:])
            # phase A: aggregation + agg@Wl, accumulate into out_buf
            for t in range(T):
                agg = agg_tile(t)
                aggT = transpose2(agg, "aggT")
                psum_o = po.tile([128, H], f32, tag="pso")
                for k in range(KC):
                    nc.tensor.matmul(psum_o[:], aggT[:, k, :], wl_sb[:, l * KC + k, :],
                                     start=(k == 0), stop=(k == KC - 1))
                nc.vector.tensor_add(out_buf[:, t, :], out_buf[:, t, :], psum_o[:])
                nc.vector.bn_stats(stats6[:, t, :], out_buf[:, t, :])
                nc.vector.bn_aggr(stats2[:, t, :], stats6[:, t, :])
            # batched stats math: T = s/sqrt(s^2 v + eps_ln), B = -mu*T
            mu = stats2[:, :, 0]
            v = stats2[:, :, 1]
            nc.vector.tensor_mul(smA[:], mu, mu)
            nc.vector.tensor_add(smA[:], smA[:], v)
            nc.scalar.activation(smB[:], smA[:], AF.Sqrt, bias=zero1[:], scale=float(H))  # r
            nc.vector.tensor_scalar_max(smB[:], smB[:], L2_EPS)
            nc.vector.reciprocal(smA[:], smB[:])                            # s
            nc.vector.tensor_mul(smB[:], smA[:], smA[:])                    # s^2
            nc.vector.tensor_mul(smB[:], smB[:], v)                         # s^2 v
            nc.scalar.activation(smB[:], smB[:], AF.Sqrt, bias=eps1[:])
            nc.vector.reciprocal(smB[:], smB[:])
            nc.vector.tensor_mul(Tt[:], smA[:], smB[:])
            nc.vector.tensor_mul(Bb[:], mu, Tt[:])
            nc.vector.tensor_scalar_mul(Bb[:], Bb[:], -1.0)
            for t in range(T):
                z = work.tile([128, H], bf16, tag="z")
                nc.scalar.activation(z[:], out_buf[:, t, :], AF.Identity,
                                     bias=Bb[:, t:t + 1], scale=Tt[:, t:t + 1])
                nc.vector.tensor_mul(z[:], z[:], gb_sb[:, l * H:(l + 1) * H])
                nc.vector.tensor_add(z[:], z[:], gb_sb[:, (L + l) * H:(L + l + 1) * H])
                nc.scalar.activation(z[:], z[:], AF.Silu, bias=zero1[:])
                nc.vector.tensor_add(h_c[:, t, :], z[:], h_c[:, t, :])
                if l < L - 1:
                    nc.sync.dma_start(bounce[t * 128:(t + 1) * 128, :], h_c[:, t, :])

        # ---- head ----
        out_sb = const.tile([128, T, 3], f32, tag="out_sb")
        for t in range(T):
            hT = transpose2(h_c[:, t, :], "hT")
            psum_o = po.tile([128, H], f32, tag="pso")
            nc.tensor.matmul(psum_o[:], ones_sb[:], bias_ap("b1"),
                             start=True, stop=False)
            for k in range(KC):
                nc.tensor.matmul(psum_o[:], hT[:, k, :], w1_sb[:, k, :],
                                 start=False, stop=(k == KC - 1))
            s_sb = work.tile([128, H], bf16, tag="s_sb")
            nc.scalar.activation(s_sb[:], psum_o[:], AF.Silu, bias=zero1[:])
            sT = transpose2(s_sb, "sT")
            psum_2 = po.tile([128, 3], f32, tag="pso")
            nc.tensor.matmul(psum_2[:], ones_sb[:], bias_ap("b2"),
                             start=True, stop=False)
            for k in range(KC):
                nc.tensor.matmul(psum_2[:], sT[:, k, :], w2_sb[:, k, :],
                                 start=False, stop=(k == KC - 1))
            nc.vector.tensor_copy(out_sb[:, t, :], psum_2[:])
        out_ap = out_ext[:].rearrange("(t p) c -> p t c", p=128)
        nc.sync.dma_start(out_ap, out_sb[:])

    nc.compile()
    return nc


def run_sharded(x, edge_index, Wp, bp, Wl, bl, Wr, ln_g, ln_b, W1, b1, W2, b2,
                W=8, trace=False, tmpdir=None):
    n_nodes, in_dim = x.shape
    L, H, _ = Wl.shape
    npc = -(-n_nodes // (W * 128)) * 128
    npad = W * npc
    KC = H // 128

    x = np.asarray(x, dtype=np.float32)
    x_pad = np.zeros((npad, in_dim), dtype=np.float32)
    x_pad[:n_nodes] = x

    sched, percore = preprocess(np.asarray(edge_index), n_nodes, W, npc)
    nc = build_program(sched, L, in_dim, H)

    def b16(a):
        return np.asarray(a, dtype=np.float32).astype(BF16)

    wl_h = np.ascontiguousarray(
        np.asarray(Wl, np.float32).reshape(L, KC, 128, H).transpose(2, 0, 1, 3)
        .reshape(128, L * KC, H)).astype(BF16)
    wr_h = np.ascontiguousarray(
        np.asarray(Wr, np.float32).reshape(L, KC, 128, H).transpose(2, 0, 1, 3)
        .reshape(128, L * KC, H)).astype(BF16)
    w1_h = np.ascontiguousarray(
        np.asarray(W1, np.float32).reshape(KC, 128, H).transpose(1, 0, 2)).astype(BF16)
    w2_h = np.ascontiguousarray(
        np.asarray(W2, np.float32).reshape(KC, 128, 3).transpose(1, 0, 2)).astype(BF16)
    biases = np.concatenate([
        np.asarray(bp, np.float32).ravel(),
        np.asarray(bl, np.float32).ravel(),
        np.asarray(b1, np.float32).ravel(),
        np.asarray(b2, np.float32).ravel()]).reshape(1, -1).astype(BF16)
    gb = np.concatenate([
        np.broadcast_to(np.asarray(ln_g, np.float32).reshape(L * H), (128, L * H)),
        np.broadcast_to(np.asarray(ln_b, np.float32).reshape(L * H), (128, L * H)),
    ], axis=1).astype(BF16)
    ones = np.ones((1, 128), dtype=BF16)
    ident = np.eye(128, dtype=np.float32).astype(BF16)
    wp_h = b16(Wp)

    in_maps = []
    for c in range(W):
        xT = np.ascontiguousarray(x_pad[c * npc:(c + 1) * npc].T).astype(BF16)
        in_maps.append(dict(
            xT=xT, idx=percore[c]["idx"], oh=percore[c]["oh"],
            inv=percore[c]["inv"], wp=wp_h, wl=wl_h, wr=wr_h, w1=w1_h, w2=w2_h,
            biases=biases, gb=gb, ones=ones, ident=ident))
    res = run_bass_kernel_spmd(nc, in_maps, core_ids=list(range(W)), trace=trace,
                               tmpdir=tmpdir)
    out = np.concatenate([res.results[c]["out"] for c in range(W)], axis=0)
    return out[:n_nodes].astype(np.float32), res


def kernel(**inputs):
    out, _ = run_sharded(
        inputs["x"], inputs["edge_index"], inputs["Wp"], inputs["bp"],
        inputs["Wl"], inputs["bl"], inputs["Wr"], inputs["ln_g"],
        inputs["ln_b"], inputs["W1"], inputs["b1"], inputs["W2"], inputs["b2"],
        W=8, trace=bool(os.environ.get("KERNEL_TRACE")))
    return out


if os.environ.get("KERNEL_TRACE"):
    _enable_axon_profile()


# revision 12
# speedup vs baseline: 1.2564x; 1.2564x over previous
"""Trainium2 Bass kernel for nn_CenterRegressor (4-layer GraphSAGE, mean-agg).

Self-contained: takes FULL inputs, shards across 8 NeuronCores internally,
returns the FULL [50000, 3] float32 output.

Design (per core, nodes sharded 8 ways, N padded 50000->50176):
  - per layer: AllGather h slices -> per-pair HBM replica h_rep (bf16)
  - edge-gather: dma_gather of 512B bf16 rows (4 SWDGE queues), edges sorted
    by (dst-tile, src-half), chunk counts padded to the max across cores so
    all 8 cores run one SPMD program
  - segment-mean: fp8 one-hot [128 slots x 128 dst] matmuls accumulate in
    PSUM; inv-degree folded into the PSUM->SBUF copy (ScalarE, per-partition
    scale); pad slots have all-zero one-hot columns
  - dense: out = agg@Wl + bl + h@Wr via DMA-transposed activations as
    stationary operands; biases as K=1 matmuls
  - epilogue: bn_stats/bn_aggr per tile + batched per-layer scalar math
    (L2-normalize + LayerNorm fold into one row-affine), SiLU on ScalarE,
    residual on VectorE
"""
import os
import sys
import types
import contextlib

import numpy as np

sys.path.insert(0, "/opt/trn_rl_repo")

import ml_dtypes  # noqa: E402
import concourse.bacc as bacc  # noqa: E402
import concourse.bass as bass  # noqa: E402
import concourse.mybir as mybir  # noqa: E402
import concourse.tile as tile  # noqa: E402
from concourse.bass_utils import run_bass_kernel_spmd  # noqa: E402
from concourse.library_config import mlp  # noqa: E402

BF16 = ml_dtypes.bfloat16
FP8 = ml_dtypes.float8_e4m3fn
AF = mybir.ActivationFunctionType

LN_EPS = 1e-5
L2_EPS = 1e-12
MAX_CHUNKS_PER_CALL = 8  # <=1024 idxs per dma_gather call


def _enable_axon_profile():
    if "antenv.axon_hooks" not in sys.modules:
        mod = types.ModuleType("antenv.axon_hooks")
        holder = [None]
        mod.set_axon_ntff_profile_hook = lambda h: holder.__setitem__(0, h)
        mod.get_axon_ntff_profile_hook = lambda: holder[0]
        sys.modules["antenv.axon_hooks"] = mod
        import antenv
        antenv.axon_hooks = mod
        try:
            from trn_agent_boot.trn_boot import _ntff_profile_via_ctypes
            mod.set_axon_ntff_profile_hook(
                _ntff_profile_via_ctypes("/opt/axon/libaxon_pjrt.so"))
        except Exception:
            pass
    import concourse.bass_utils as bu
    bu.upload_artifacts = lambda tmpdir: f"file://{tmpdir}"


def _idx_layout(flat):
    """int16 [S] -> [128, S/16] wrapped in 16 partitions, replicated x8."""
    s = flat.shape[0]
    assert s % 16 == 0
    return np.tile(flat.reshape(s // 16, 16).T, (8, 1)).astype(np.int16)


def preprocess(edge_index, n_nodes, W, npc):
    """Build the uniform SPMD schedule + per-core gather/one-hot data.

    Returns (sched, percore) where sched is compile-time (identical for all
    cores) and percore is a list of dicts of numpy arrays.
    """
    npad = W * npc
    halfr = npad // 2
    T = npc // 128
    src = np.asarray(edge_index[0], dtype=np.int64)
    dst = np.asarray(edge_index[1], dtype=np.int64)

    deg = np.bincount(dst, minlength=npad).astype(np.float64)
    inv_deg = (1.0 / np.maximum(deg, 1.0)).astype(np.float32)

    core_of = dst // npc
    # per (core, tile, half) edge lists
    counts = np.zeros((W, T, 2), dtype=np.int64)
    per_cth = {}
    for c in range(W):
        m = core_of == c
        s_c = src[m]
        dl = dst[m] - c * npc
        t_c = dl // 128
        p_c = dl % 128
        hf_c = (s_c >= halfr).astype(np.int64)
        key = t_c * 2 + hf_c
        order = np.argsort(key, kind="stable")
        s_c, p_c, t_c, hf_c, key = (a[order] for a in (s_c, p_c, t_c, hf_c, key))
        bounds = np.searchsorted(key, np.arange(2 * T + 1))
        for t in range(T):
            for hf in range(2):
                lo, hi = bounds[t * 2 + hf], bounds[t * 2 + hf + 1]
                per_cth[(c, t, hf)] = (s_c[lo:hi] - hf * halfr, p_c[lo:hi])
                counts[c, t, hf] = hi - lo

    # uniform chunk counts across cores
    K = np.maximum(np.ceil(counts / 128.0).astype(np.int64).max(axis=0), 0)
    K[:, 0] = np.maximum(K[:, 0], 1)  # >=1 chunk so PSUM gets a start matmul

    # global chunk order: all lo chunks (by tile), then all hi chunks (by tile)
    # cth_off[hf][t] = first global chunk index of (t, hf)
    nlo = int(K[:, 0].sum())
    nhi = int(K[:, 1].sum())
    total_chunks = nlo + nhi
    total_slots = total_chunks * 128
    cth_off = np.zeros((2, T + 1), dtype=np.int64)
    cth_off[0, 1:] = np.cumsum(K[:, 0])
    cth_off[1, 1:] = nlo + np.cumsum(K[:, 1])
    cth_off[1, 0] = nlo

    # calls: runs of <=8 chunks within each half, spanning tiles
    calls = []  # (chunk0, n_chunks, half)
    for hf in range(2):
        lo, hi = (0, nlo) if hf == 0 else (nlo, nlo + nhi)
        c = lo
        while c < hi:
            n = min(MAX_CHUNKS_PER_CALL, hi - c)
            calls.append((c, n, hf))
            c += n

    sched = dict(W=W, npc=npc, npad=npad, halfr=halfr, T=T,
                 K=K, cth_off=cth_off, total_chunks=total_chunks,
                 total_slots=total_slots, calls=calls)

    percore = []
    for c in range(W):
        idx_flat = np.zeros(total_slots, dtype=np.int16)
        oh = np.zeros((128, total_chunks * 128), dtype=FP8)
        for t in range(T):
            for hf in range(2):
                srel, p = per_cth[(c, t, hf)]
                off = int(cth_off[hf, t]) * 128
                n = srel.shape[0]
                idx_flat[off:off + n] = srel.astype(np.int16)
                sl = np.arange(n) + off
                # slot s lives at partition s%128, chunk s//128
                oh[sl % 128, (sl // 128) * 128 + p] = 1.0
        inv_sb = inv_deg[c * npc:(c + 1) * npc].reshape(T, 128).T.copy()  # [128,T]
        percore.append(dict(idx=_idx_layout(idx_flat), oh=oh, inv=inv_sb))
    return sched, percore


def build_program(sched, L, in_dim, H):
    W, npc, T = sched["W"], sched["npc"], sched["T"]
    npad, halfr = sched["npad"], sched["halfr"]
    K, cth_off, calls = sched["K"], sched["cth_off"], sched["calls"]
    total_chunks, total_slots = sched["total_chunks"], sched["total_slots"]
    KC = H // 128  # feature chunks (2)
    ohmax = int(max((sched["K"][t, 0] + sched["K"][t, 1]) for t in range(sched["T"])))

    nc = bacc.Bacc("TRN2", debug=True, num_swdge_queues=4)
    f32, bf16, fp8, i16 = (mybir.dt.float32, mybir.dt.bfloat16,
                           mybir.dt.float8e4, mybir.dt.int16)

    # ---- external IO ----
    xT_in = nc.dram_tensor("xT", [in_dim, npc], bf16, kind="ExternalInput")
    idx_in = nc.dram_tensor("idx", [128, total_slots // 16], i16, kind="ExternalInput")
    oh_in = nc.dram_tensor("oh", [128, total_chunks * 128], fp8, kind="ExternalInput")
    inv_in = nc.dram_tensor("inv", [128, T], f32, kind="ExternalInput")
    wp_in = nc.dram_tensor("wp", [in_dim, H], bf16, kind="ExternalInput")
    wl_in = nc.dram_tensor("wl", [128, L * KC, H], bf16, kind="ExternalInput")
    wr_in = nc.dram_tensor("wr", [128, L * KC, H], bf16, kind="ExternalInput")
    w1_in = nc.dram_tensor("w1", [128, KC, H], bf16, kind="ExternalInput")
    w2_in = nc.dram_tensor("w2", [128, KC, 3], bf16, kind="ExternalInput")
    bias_in = nc.dram_tensor("biases", [1, (L + 2) * H + 3], bf16, kind="ExternalInput")
    gb_in = nc.dram_tensor("gb", [128, 2 * L * H], bf16, kind="ExternalInput")
    ones_in = nc.dram_tensor("ones", [1, 128], bf16, kind="ExternalInput")
    ident_in = nc.dram_tensor("ident", [128, 128], bf16, kind="ExternalInput")
    out_ext = nc.dram_tensor("out", [npc, 3], f32, kind="ExternalOutput")

    # ---- internal DRAM ----
    bounce = nc.dram_tensor("bounce", [npc, H], fp8)
    h_rep = nc.dram_tensor("h_rep", [npad, H], fp8,
                           addr_space="Shared" if W > 1 else "Local")

    with contextlib.ExitStack() as ctx:
        tc = ctx.enter_context(tile.TileContext(nc))
        const = ctx.enter_context(tc.tile_pool(name="const", bufs=1))
        gpool = ctx.enter_context(tc.tile_pool(name="gath", bufs=16))
        ohpool = ctx.enter_context(tc.tile_pool(name="ohp", bufs=4))
        work = ctx.enter_context(tc.tile_pool(name="work", bufs=4))
        pa = ctx.enter_context(tc.tile_pool(name="pa", bufs=2, space="PSUM"))
        pt = ctx.enter_context(tc.tile_pool(name="pt", bufs=3, space="PSUM"))
        po = ctx.enter_context(tc.tile_pool(name="po", bufs=3, space="PSUM"))

        nc.gpsimd.load_library(mlp)

        def load_const(name, dram, shape, dt):
            t = const.tile(shape, dt, tag=name)
            nc.sync.dma_start(t[:], dram[:])
            return t

        idx_sb = load_const("idx", idx_in, [128, total_slots // 16], i16)
        inv_sb = load_const("inv", inv_in, [128, T], f32)
        xT_sb = load_const("xT", xT_in, [in_dim, npc], bf16)
        wp_sb = load_const("wp", wp_in, [in_dim, H], bf16)
        wl_sb = load_const("wl", wl_in, [128, L * KC, H], bf16)
        wr_sb = load_const("wr", wr_in, [128, L * KC, H], bf16)
        w1_sb = load_const("w1", w1_in, [128, KC, H], bf16)
        w2_sb = load_const("w2", w2_in, [128, KC, 3], bf16)
        bias_sb = load_const("biases", bias_in, [1, (L + 2) * H + 3], bf16)
        gb_sb = load_const("gb", gb_in, [128, 2 * L * H], bf16)
        ones_sb = load_const("ones", ones_in, [1, 128], bf16)
        ident_sb = load_const("ident", ident_in, [128, 128], bf16)

        zero1 = const.tile([128, 1], f32, tag="zero1")
        nc.vector.memset(zero1[:], 0.0)
        eps1 = const.tile([128, 1], f32, tag="eps1")
        nc.vector.memset(eps1[:], LN_EPS)

        h_c = const.tile([128, T, H], bf16, tag="h_c")
        out_buf = const.tile([128, T, H], bf16, tag="out_buf")
        stats6 = const.tile([128, T, 6], f32, tag="stats6")
        stats2 = const.tile([128, T, 2], f32, tag="stats2")
        smA = const.tile([128, T], f32, tag="smA")
        smB = const.tile([128, T], f32, tag="smB")
        Tt = const.tile([128, T], f32, tag="Tt")
        Bb = const.tile([128, T], f32, tag="Bb")

        def bias_ap(which):
            # biases layout: [bp(H) | bl0..bl3 (L*H) | b1(H) | b2(3)]
            if which == "bp":
                return bias_sb[:, 0:H]
            if which.startswith("bl"):
                l = int(which[2:])
                return bias_sb[:, H + l * H: H + (l + 1) * H]
            if which == "b1":
                return bias_sb[:, (L + 1) * H:(L + 2) * H]
            return bias_sb[:, (L + 2) * H:(L + 2) * H + 3]

        # issue gather calls lazily; chunk c served by call c//8 slot c%8
        gt_bufs = {}

        def ensure_call(ci):
            if ci in gt_bufs:
                return gt_bufs[ci]
            (c0, nch, hf) = calls[ci]
            gt = gpool.tile([128, MAX_CHUNKS_PER_CALL, H], fp8, tag="gt")
            nidx = nch * 128
            slot_off = c0 * 128
            src_ap = h_rep[0:halfr, :] if hf == 0 else h_rep[halfr:npad, :]
            nc.gpsimd.dma_gather(
                gt[:, 0:nch, :], src_ap,
                idx_sb[:, slot_off // 16:(slot_off + nidx) // 16],
                nidx, nidx, H,
                single_packet=False, queue_num=ci % 4)
            gt_bufs[ci] = gt
            return gt

        call_of_chunk = {}
        for ci, (c0, nch, hf) in enumerate(calls):
            for j in range(nch):
                call_of_chunk[c0 + j] = (ci, j)

        def agg_tile(t):
            """Gather + one-hot matmul + inv-deg scale -> agg [128,H] bf16."""
            chunks = (list(range(int(cth_off[0, t]), int(cth_off[0, t + 1]))) +
                      list(range(int(cth_off[1, t]), int(cth_off[1, t + 1]))))
            ktot = len(chunks)
            oh_t = ohpool.tile([128, ohmax * 128], fp8, tag="oh")
            lo0, lo1 = int(cth_off[0, t]), int(cth_off[0, t + 1])
            hi0, hi1 = int(cth_off[1, t]), int(cth_off[1, t + 1])
            nlo_t = lo1 - lo0
            nc.sync.dma_start(oh_t[:, 0:nlo_t * 128], oh_in[:, lo0 * 128:lo1 * 128])
            nc.sync.dma_start(oh_t[:, nlo_t * 128:ktot * 128],
                              oh_in[:, hi0 * 128:hi1 * 128])
            psum_a = pa.tile([128, H], f32, tag="psa")
            mms = []  # (ci, j, i, pair?)
            i = 0
            while i < ktot:
                ci, j = call_of_chunk[chunks[i]]
                if (i + 1 < ktot and j + 1 < MAX_CHUNKS_PER_CALL
                        and chunks[i + 1] == chunks[i] + 1
                        and call_of_chunk[chunks[i + 1]] == (ci, j + 1)):
                    mms.append((ci, j, i, True))
                    i += 2
                else:
                    mms.append((ci, j, i, False))
                    i += 1
            for n, (ci, j, i, pair) in enumerate(mms):
                gt = ensure_call(ci)
                st, sp = (n == 0), (n == len(mms) - 1)
                if pair:
                    oh_ap = oh_t[:, i * 128:(i + 2) * 128].rearrange(
                        "p (k d) -> p k d", k=2)
                    nc.tensor.matmul(
                        psum_a[:], oh_ap, gt[:, j:j + 2, :],
                        start=st, stop=sp,
                        perf_mode=mybir.MatmulPerfMode.DoubleRow)
                else:
                    nc.tensor.matmul(
                        psum_a[:], oh_t[:, i * 128:(i + 1) * 128], gt[:, j, :],
                        start=st, stop=sp)
            agg = work.tile([128, H], bf16, tag="agg")
            nc.scalar.activation(agg[:], psum_a[:], AF.Copy, scale=inv_sb[:, t:t + 1])
            return agg

        def transpose2(src_ap, tag):
            tt = work.tile([128, KC, 128], bf16, tag=tag)
            for k in range(KC):
                pst = pt.tile([128, 128], bf16, tag="pst")
                nc.tensor.transpose(pst[:], src_ap[:, k * 128:(k + 1) * 128],
                                    ident_sb[:])
                if k % 2 == 0:
                    nc.scalar.activation(tt[:, k, :], pst[:], AF.Copy)
                else:
                    nc.vector.tensor_copy(tt[:, k, :], pst[:])
            return tt

        AGQ = 4 if T % 4 == 0 else 1
        h_rep_v = h_rep[:].rearrange("(w n) h -> w n h", w=W)

        def allgather():
            if W == 1:
                nc.sync.dma_start(h_rep[:], bounce[:])
                return
            q = npc // AGQ
            for i in range(AGQ):
                nc.gpsimd.collective_compute(
                    "AllGather", mybir.AluOpType.bypass,
                    replica_groups=[list(range(W))],
                    ins=[bounce[i * q:(i + 1) * q, :].opt()],
                    outs=[h_rep_v[:, i * q:(i + 1) * q, :].opt()])

        # ---- input projection ----
        for t in range(T):
            psum_o = po.tile([128, H], f32, tag="pso")
            nc.tensor.matmul(psum_o[:], ones_sb[:], bias_ap("bp"),
                             start=True, stop=False)
            nc.tensor.matmul(psum_o[:], xT_sb[:, t * 128:(t + 1) * 128], wp_sb[:],
                             start=False, stop=True)
            nc.scalar.activation(h_c[:, t, :], psum_o[:], AF.Copy)
            h8 = work.tile([128, H], fp8, tag="h8")
            nc.scalar.activation(h8[:], psum_o[:], AF.Copy)
            nc.sync.dma_start(bounce[t * 128:(t + 1) * 128, :], h8[:])

        # ---- layers ----
        for l in range(L):
            gt_bufs.clear()  # per-layer gather-call memo
            with nc.named_scope(f"ag{l}"):
                allgather()
            # phase H: AllGather-independent dense part (bias + h@Wr)
            for t in range(T):
                hT = transpose2(h_c[:, t, :], "hT")
                psum_h = po.tile([128, H], f32, tag="pso")
                nc.tensor.matmul(psum_h[:], ones_sb[:], bias_ap(f"bl{l}"),
                                 start=True, stop=False)
                for k in range(KC):
                    nc.tensor.matmul(psum_h[:], hT[:, k, :], wr_sb[:, l * KC + k, :],
                                     start=False, stop=(k == KC - 1))
                nc.vector.tensor_copy(out_buf[:, t, :], psum_h[:])
            # phase A: aggregation + agg@Wl, accumulate into out_buf
            for t in range(T):
                agg = agg_tile(t)
                aggT = transpose2(agg, "aggT")
                psum_o = po.tile([128, H], f32, tag="pso")
                for k in range(KC):
                    nc.tensor.matmul(psum_o[:], aggT[:, k, :], wl_sb[:, l * KC + k, :],
                                     start=(k == 0), stop=(k == KC - 1))
                nc.vector.tensor_add(out_buf[:, t, :], out_buf[:, t, :], psum_o[:])
                nc.vector.bn_stats(stats6[:, t, :], out_buf[:, t, :])
                nc.vector.bn_aggr(stats2[:, t, :], stats6[:, t, :])
            # batched stats math: T = s/sqrt(s^2 v + eps_ln), B = -mu*T
            mu = stats2[:, :, 0]
            v = stats2[:, :, 1]
            nc.vector.tensor_mul(smA[:], mu, mu)
            nc.vector.tensor_add(smA[:], smA[:], v)
            nc.scalar.activation(smB[:], smA[:], AF.Sqrt, bias=zero1[:], scale=float(H))  # r
            nc.vector.tensor_scalar_max(smB[:], smB[:], L2_EPS)
            nc.vector.reciprocal(smA[:], smB[:])                            # s
            nc.vector.tensor_mul(smB[:], smA[:], smA[:])                    # s^2
            nc.vector.tensor_mul(smB[:], smB[:], v)                         # s^2 v
            nc.scalar.activation(smB[:], smB[:], AF.Sqrt, bias=eps1[:])
            nc.vector.reciprocal(smB[:], smB[:])
            nc.vector.tensor_mul(Tt[:], smA[:], smB[:])
            nc.vector.tensor_mul(Bb[:], mu, Tt[:])
            nc.vector.tensor_scalar_mul(Bb[:], Bb[:], -1.0)
            for t in range(T):
                z = work.tile([128, H], bf16, tag="z")
                nc.scalar.activation(z[:], out_buf[:, t, :], AF.Identity,
                                     bias=Bb[:, t:t + 1], scale=Tt[:, t:t + 1])
                nc.vector.tensor_mul(z[:], z[:], gb_sb[:, l * H:(l + 1) * H])
                nc.vector.tensor_add(z[:], z[:], gb_sb[:, (L + l) * H:(L + l + 1) * H])
                nc.scalar.activation(z[:], z[:], AF.Silu, bias=zero1[:])
                nc.vector.tensor_add(h_c[:, t, :], z[:], h_c[:, t, :])
                if l < L - 1:
                    h8 = work.tile([128, H], fp8, tag="h8")
                    nc.scalar.activation(h8[:], h_c[:, t, :], AF.Copy)
                    nc.sync.dma_start(bounce[t * 128:(t + 1) * 128, :], h8[:])

        # ---- head ----
        out_sb = const.tile([128, T, 3], f32, tag="out_sb")
        for t in range(T):
            hT = transpose2(h_c[:, t, :], "hT")
            psum_o = po.tile([128, H], f32, tag="pso")
            nc.tensor.matmul(psum_o[:], ones_sb[:], bias_ap("b1"),
                             start=True, stop=False)
            for k in range(KC):
                nc.tensor.matmul(psum_o[:], hT[:, k, :], w1_sb[:, k, :],
                                 start=False, stop=(k == KC - 1))
            s_sb = work.tile([128, H], bf16, tag="s_sb")
            nc.scalar.activation(s_sb[:], psum_o[:], AF.Silu, bias=zero1[:])
            sT = transpose2(s_sb, "sT")
            psum_2 = po.tile([128, 3], f32, tag="pso")
            nc.tensor.matmul(psum_2[:], ones_sb[:], bias_ap("b2"),
                             start=True, stop=False)
            for k in range(KC):
                nc.tensor.matmul(psum_2[:], sT[:, k, :], w2_sb[:, k, :],
                                 start=False, stop=(k == KC - 1))
            nc.vector.tensor_copy(out_sb[:, t, :], psum_2[:])
        out_ap = out_ext[:].rearrange("(t p) c -> p t c", p=128)
        nc.sync.dma_start(out_ap, out_sb[:])

    nc.compile()
    return nc


def run_sharded(x, edge_index, Wp, bp, Wl, bl, Wr, ln_g, ln_b, W1, b1, W2, b2,
                W=8, trace=False, tmpdir=None):
    n_nodes, in_dim = x.shape
    L, H, _ = Wl.shape
    npc = -(-n_nodes // (W * 128)) * 128
    npad = W * npc
    KC = H // 128

    x = np.asarray(x, dtype=np.float32)
    x_pad = np.zeros((npad, in_dim), dtype=np.float32)
    x_pad[:n_nodes] = x

    sched, percore = preprocess(np.asarray(edge_index), n_nodes, W, npc)
    nc = build_program(sched, L, in_dim, H)

    def b16(a):
        return np.asarray(a, dtype=np.float32).astype(BF16)

    wl_h = np.ascontiguousarray(
        np.asarray(Wl, np.float32).reshape(L, KC, 128, H).transpose(2, 0, 1, 3)
        .reshape(128, L * KC, H)).astype(BF16)
    wr_h = np.ascontiguousarray(
        np.asarray(Wr, np.float32).reshape(L, KC, 128, H).transpose(2, 0, 1, 3)
        .reshape(128, L * KC, H)).astype(BF16)
    w1_h = np.ascontiguousarray(
        np.asarray(W1, np.float32).reshape(KC, 128, H).transpose(1, 0, 2)).astype(BF16)
    w2_h = np.ascontiguousarray(
        np.asarray(W2, np.float32).reshape(KC, 128, 3).transpose(1, 0, 2)).astype(BF16)
    biases = np.concatenate([
        np.asarray(bp, np.float32).ravel(),
        np.asarray(bl, np.float32).ravel(),
        np.asarray(b1, np.float32).ravel(),
        np.asarray(b2, np.float32).ravel()]).reshape(1, -1).astype(BF16)
    gb = np.concatenate([
        np.broadcast_to(np.asarray(ln_g, np.float32).reshape(L * H), (128, L * H)),
        np.broadcast_to(np.asarray(ln_b, np.float32).reshape(L * H), (128, L * H)),
    ], axis=1).astype(BF16)
    ones = np.ones((1, 128), dtype=BF16)
    ident = np.eye(128, dtype=np.float32).astype(BF16)
    wp_h = b16(Wp)

    in_maps = []
    for c in range(W):
        xT = np.ascontiguousarray(x_pad[c * npc:(c + 1) * npc].T).astype(BF16)
        in_maps.append(dict(
            xT=xT, idx=percore[c]["idx"], oh=percore[c]["oh"],
            inv=percore[c]["inv"], wp=wp_h, wl=wl_h, wr=wr_h, w1=w1_h, w2=w2_h,
            biases=biases, gb=gb, ones=ones, ident=ident))
    res = run_bass_kernel_spmd(nc, in_maps, core_ids=list(range(W)), trace=trace,
                               tmpdir=tmpdir)
    out = np.concatenate([res.results[c]["out"] for c in range(W)], axis=0)
    return out[:n_nodes].astype(np.float32), res


def kernel(**inputs):
    out, _ = run_sharded(
        inputs["x"], inputs["edge_index"], inputs["Wp"], inputs["bp"],
        inputs["Wl"], inputs["bl"], inputs["Wr"], inputs["ln_g"],
        inputs["ln_b"], inputs["W1"], inputs["b1"], inputs["W2"], inputs["b2"],
        W=8, trace=bool(os.environ.get("KERNEL_TRACE")))
    return out


if os.environ.get("KERNEL_TRACE"):
    _enable_axon_profile()
